# revision 1
# baseline (speedup 1.0000x reference)
"""GAT attention head (nn_AttnHead) on 8 Trainium2 NeuronCores.

Strategy (row-sharded, per sharding hint):
  - Core c owns query rows i in [c*512, (c+1)*512) for both batches.
  - Each core projects its own seq slice -> seq_fts (64ch), computes f1
    locally; seq_fts^T, ones, f2 are AllGathered so every core has all keys.
  - Attention is computed in TRANSPOSED layout [j (keys, partitions),
    i (queries, free)]: lrelu(f1[i]+f2[j]) on ACT (Lrelu, alpha=0.01),
    bias_mat row-block is PE-transposed into PSUM and added on DVE,
    exp on ACT, then one PE matmul per (b, j-chunk) with lhsT =
    [seq_fts | ones] accumulates numerator AND softmax denominator.
  - BatchNorm batch stats via a tiny AllReduce; normalize + ELU on chip;
    PE-transpose the [64, i] result back to [i, 64] rows and DMA out.

Implementation notes:
  - Built with Bacc (not raw Bass): its generate_event_semaphores pass
    splits multi-sem waits that exceed per-instruction HW wait capacity.
  - Softmax rows are shift-invariant, so f1[i] is dropped from the
    logits (cancels in num/den); f2[j] rides the ACT exp bias operand,
    and lrelu is expressed as v + (1-slope)*relu(-v) because leaky_relu
    and exp share no ACT table set (Relu does) - avoids table thrash.
  - Aggregation matmuls run as float32r (1 cycle/row vs 4 for fp32).
  - Bias DMAs are split across both HWDGE issuers (sync + scalar) with
    deep pool buffering for queue-level parallelism; relu work is split
    ACT/DVE to balance engine occupancy.
"""

import numpy as np

import concourse.bass as bass
import concourse.bacc as bacc
import concourse.tile as tile
from concourse import mybir
from concourse.bass_utils import run_bass_kernel_spmd

B, N, F, O = 2, 4096, 256, 64
P = 128
R = 8                 # cores
NL = N // R           # 512 local query rows per core
NB = NL // P          # 4 row blocks
JC = N // P           # 32 key chunks of 128
JG = JC // NB         # 8 key groups of 512
AGW = O + 3           # AllGather payload: [sfT(64) | ones | f2 | -f2]
SLOPE = 0.01
EPS = 1e-5
f32 = mybir.dt.float32
f32r = mybir.dt.float32r
AFT = mybir.ActivationFunctionType
ALU = mybir.AluOpType

_CACHE = {}


def _build_program(n_reps=1, dma_only=False, f32r_agg=False, dve_lrelu_mod=0, dma8=False, no_cc=False, no_bias_dma=False, f32r_tr=False, ilv=False):
    key = (n_reps, dma_only, f32r_agg, dve_lrelu_mod, dma8, no_cc, no_bias_dma, f32r_tr, ilv)
    if key in _CACHE:
        return _CACHE[key]

    nc = bacc.Bacc("TRN2", target_bir_lowering=False, debug=False, num_devices=R)

    seq_in = nc.dram_tensor("seq_loc", [B, NL, F], f32, kind="ExternalInput").ap()
    bias_in = nc.dram_tensor("bias_loc", [B, NL, N], f32, kind="ExternalInput").ap()
    w1t_in = nc.dram_tensor("w1t", [F, O], f32, kind="ExternalInput").ap()
    w2_in = nc.dram_tensor("w2c", [O, 1], f32, kind="ExternalInput").ap()
    w3_in = nc.dram_tensor("w3c", [O, 1], f32, kind="ExternalInput").ap()
    gam_in = nc.dram_tensor("gamma_c", [O, 1], f32, kind="ExternalInput").ap()
    bet_in = nc.dram_tensor("beta_c", [O, 1], f32, kind="ExternalInput").ap()
    sc_in = nc.dram_tensor("scalars", [1, 2], f32, kind="ExternalInput").ap()
    out_ext = nc.dram_tensor("out_loc", [B, NL, O], f32, kind="ExternalOutput").ap()

    ag_in = nc.dram_tensor("ag_in", [B * NL, AGW], f32)
    ag_out = nc.dram_tensor("ag_out", [R * B * NL, AGW], f32, addr_space="Shared")
    st_in = nc.dram_tensor("st_in", [O, 2], f32)
    st_out = nc.dram_tensor("st_out", [R * O, 2], f32, addr_space="Shared")

    ident_d = nc.inline_tensor(np.eye(P, dtype=np.float32), name="ident")
    rg = [list(range(R))]

    with tile.TileContext(nc, num_cores=R) as tc:
        with (
            tc.tile_pool(name="consts", bufs=1) as consts,
            tc.tile_pool(name="perb", bufs=2) as perb,
            tc.tile_pool(name="persist", bufs=1) as persist,
            tc.tile_pool(name="biasg", bufs=9) as biasg,
            tc.tile_pool(name="work", bufs=4) as work,
            tc.tile_pool(name="stage", bufs=8) as stage,
            tc.tile_pool(name="tailp", bufs=2) as tailp,
            tc.tile_pool(name="ps_big", bufs=3, space="PSUM") as ps_big,
            tc.tile_pool(name="ps_agg", bufs=2, space="PSUM") as ps_agg,
            tc.tile_pool(name="ps_proj", bufs=1, space="PSUM") as ps_proj,
            tc.tile_pool(name="ps_misc", bufs=2, space="PSUM") as ps_misc,
        ):
            # ---------- constants ----------
            ident = consts.tile([P, P], f32)
            nc.sync.dma_start(out=ident, in_=ident_d.ap())
            w1a = consts.tile([P, O], f32)
            nc.sync.dma_start(out=w1a, in_=w1t_in[0:P, :])
            w1b = consts.tile([P, O], f32)
            nc.sync.dma_start(out=w1b, in_=w1t_in[P:F, :])
            w2c = consts.tile([O, 1], f32)
            nc.sync.dma_start(out=w2c, in_=w2_in)
            w3c = consts.tile([O, 1], f32)
            nc.sync.dma_start(out=w3c, in_=w3_in)
            gam = consts.tile([O, 1], f32)
            nc.sync.dma_start(out=gam, in_=gam_in)
            bet = consts.tile([O, 1], f32)
            nc.sync.dma_start(out=bet, in_=bet_in)
            b2t = consts.tile([1, 1], f32)
            nc.sync.dma_start(out=b2t, in_=sc_in[0:1, 0:1])
            b3r = consts.tile([P, 1], f32)
            nc.gpsimd.dma_start(
                out=b3r,
                in_=bass.AP(tensor=sc_in.tensor, offset=1, ap=[[0, P], [1, 1]]),
            )
            # constants built on ACT (so matmuls reading them wait on ACT only)
            ones_r = consts.tile([1, P], f32)
            nc.scalar.activation(ones_r, ident[0:1, :], AFT.Copy, bias=1.0, scale=0.0)
            ones_o = consts.tile([1, O], f32)
            nc.scalar.activation(ones_o, ident[0:1, 0:O], AFT.Copy, bias=1.0, scale=0.0)
            eps_t = consts.tile([O, 1], f32)
            nc.scalar.activation(eps_t, ident[0:O, 0:1], AFT.Copy, bias=EPS, scale=0.0)
            b3n = consts.tile([P, 1], f32)
            nc.vector.tensor_scalar_mul(b3n, b3r, -1.0)

            valsT = persist.tile([O, B * NL], f32, tag="valsT")

            def _rep_body():
                # ---------- phase A: projection + AllGather ----------
                stgall = persist.tile([P, B, NB, AGW], f32, tag="stgall")
                f1_reps = []
                for b in range(B):
                    ps_sf = ps_proj.tile([O, NL], f32, tag="ps_sf")
                    for nb in range(NB):
                        seq_t = stage.tile([P, F], f32, tag="seq_t")
                        nc.sync.dma_start(
                            out=seq_t, in_=seq_in[b, nb * P:(nb + 1) * P, :]
                        )
                        ps_sT = ps_misc.tile([P, 2, P], f32, tag="pmisc")
                        nc.tensor.transpose(ps_sT[:, 0, :], seq_t[:, 0:P], ident)
                        nc.tensor.transpose(ps_sT[:, 1, :], seq_t[:, P:F], ident)
                        sT = stage.tile([P, 2, P], f32, tag="sT")
                        nc.vector.tensor_copy(sT, ps_sT)
                        nc.tensor.matmul(
                            ps_sf[:, nb * P:(nb + 1) * P], lhsT=w1a, rhs=sT[:, 0, :],
                            start=True, stop=False,
                        )
                        nc.tensor.matmul(
                            ps_sf[:, nb * P:(nb + 1) * P], lhsT=w1b, rhs=sT[:, 1, :],
                            start=False, stop=True,
                        )
                    sf_loc = perb.tile([O, NL], f32, tag="sf_loc")
                    nc.vector.tensor_copy(sf_loc, ps_sf)

                    ps_f1 = ps_misc.tile([1, NL], f32, tag="pmisc")
                    nc.tensor.matmul(ps_f1, lhsT=w2c, rhs=sf_loc, start=True, stop=True)
                    f1row = stage.tile([1, NL], f32, tag="f1row")
                    nc.scalar.activation(f1row, ps_f1, AFT.Identity, bias=b2t)
                    ps_rep = ps_misc.tile([P, NL], f32, tag="pmisc")
                    nc.tensor.matmul(ps_rep, lhsT=ones_r, rhs=f1row, start=True, stop=True)
                    f1_rep = perb.tile([P, NL], f32, tag="f1_rep")
                    nc.vector.tensor_copy(f1_rep, ps_rep)
                    f1_reps.append(f1_rep)

                    for nb in range(NB):
                        ps_sfT = ps_misc.tile([P, O], f32, tag="pmisc")
                        nc.tensor.transpose(
                            ps_sfT, sf_loc[:, nb * P:(nb + 1) * P], ident[0:O, 0:O]
                        )
                        ps_f2T = ps_misc.tile([P, 1], f32, tag="pmisc")
                        nc.tensor.matmul(
                            ps_f2T, lhsT=sf_loc[:, nb * P:(nb + 1) * P], rhs=w3c,
                            start=True, stop=True,
                        )
                        nc.vector.tensor_copy(stgall[:, b, nb, 0:O], ps_sfT)
                        nc.scalar.activation(
                            stgall[:, b, nb, O:O + 1], ident[:, 0:1],
                            AFT.Copy, bias=1.0, scale=0.0,
                        )
                        nc.scalar.activation(
                            stgall[:, b, nb, O + 1:O + 2], ps_f2T, AFT.Identity, bias=b3r
                        )
                        nc.scalar.activation(
                            stgall[:, b, nb, O + 2:O + 3], ps_f2T, AFT.Identity,
                            bias=b3n, scale=-1.0,
                        )
                # one DMA -> ag_in (single queue sem for the collective to wait on)
                nc.sync.dma_start(
                    out=bass.AP(
                        tensor=ag_in.ap().tensor, offset=0,
                        ap=[[AGW, P], [NL * AGW, B], [P * AGW, NB], [1, AGW]],
                    ),
                    in_=stgall,
                )
                if not no_cc:
                    nc.gpsimd.collective_compute(
                        "AllGather", ALU.bypass, replica_groups=rg,
                        ins=[ag_in.ap()], outs=[ag_out.ap()],
                    )

                # ---------- phase B: attention main loop ----------
                ps_ags = []
                if ilv:
                    # batch-interleaved main loop: two independent accumulation
                    # chains give the scheduler more gap-filling freedom
                    sfalls = []
                    for b in range(B):
                        sfall = perb.tile([P, R, NB, AGW], f32, tag="sfall")
                        for rank in range(R):
                            nc.sync.dma_start(
                                out=sfall[:, rank, :, :],
                                in_=bass.AP(
                                    tensor=ag_out.ap().tensor,
                                    offset=(rank * B * NL + b * NL) * AGW,
                                    ap=[[AGW, P], [P * AGW, NB], [1, AGW]],
                                ),
                            )
                        sfalls.append(sfall)
                        ps_ag_i = ps_agg.tile([O + 1, NL], f32, tag="agg")
                        ps_ags.append(ps_ag_i)
                    bgas = [None, None]
                    dma_engs = [nc.sync, nc.scalar, nc.sync, nc.scalar]
                    for jc in range(JC):
                        jg, jo = jc // NB, jc % NB
                        for b in range(B):
                            if jo == 0:
                                bga = biasg.tile([P, NB, 512], f32, tag="biasg")
                                for ib in range(NB):
                                    for hh in range(2):
                                        dma_engs[(2 * ib + hh) % 4].dma_start(
                                            out=bga[:, ib, hh * 256:(hh + 1) * 256],
                                            in_=bias_in[b, ib * P:(ib + 1) * P,
                                                        jg * 512 + hh * 256:
                                                        jg * 512 + (hh + 1) * 256],
                                        )
                                bgas[b] = bga
                            rank, nb_r = jc // NB, jc % NB
                            ps_bT = ps_big.tile([P, NL], f32, tag="biasT")
                            for ib in range(NB):
                                nc.tensor.transpose(
                                    ps_bT[:, ib * P:(ib + 1) * P],
                                    bgas[b][:, ib, jo * P:(jo + 1) * P], ident,
                                )
                            f2c = sfalls[b][:, rank, nb_r, O + 1:O + 2]
                            nf2c = sfalls[b][:, rank, nb_r, O + 2:O + 3]
                            w = work.tile([P, NL], f32, tag="w")
                            if jc % 16 < 9:
                                t = work.tile([P, NL], f32, tag="t")
                                nc.vector.tensor_scalar(
                                    t, f1_reps[b], f2c, -(1.0 - SLOPE),
                                    ALU.add, ALU.mult,
                                )
                                nc.vector.scalar_tensor_tensor(
                                    w, t, 0.0, ps_bT, ALU.max, ALU.add
                                )
                            else:
                                r = work.tile([P, NL], f32, tag="r")
                                nc.scalar.activation(
                                    r, f1_reps[b], AFT.Relu, bias=nf2c, scale=-1.0
                                )
                                nc.vector.scalar_tensor_tensor(
                                    w, r, 1.0 - SLOPE, ps_bT, ALU.mult, ALU.add
                                )
                            edt = f32r if f32r_agg else f32
                            e = work.tile([P, NL], edt, tag="e")
                            nc.scalar.activation(e, w, AFT.Exp, bias=f2c)
                            if f32r_agg:
                                nc.tensor.matmul(
                                    ps_ags[b],
                                    lhsT=sfalls[b][:, rank, nb_r, 0:O + 1].bitcast(f32r),
                                    rhs=e,
                                    start=(jc == 0), stop=(jc == JC - 1),
                                )
                            else:
                                nc.tensor.matmul(
                                    ps_ags[b],
                                    lhsT=sfalls[b][:, rank, nb_r, 0:O + 1], rhs=e,
                                    start=(jc == 0), stop=(jc == JC - 1),
                                )
                elif True:
                    pass
                for b in range(B) if not ilv else []:
                    # all 32 [sfT | ones | f2] chunks for this batch: ONE DMA
                    sfall = perb.tile([P, R, NB, AGW], f32, tag="sfall")
                    for rank in range(R):
                        nc.sync.dma_start(
                            out=sfall[:, rank, :, :],
                            in_=bass.AP(
                                tensor=ag_out.ap().tensor,
                                offset=(rank * B * NL + b * NL) * AGW,
                                ap=[[AGW, P], [P * AGW, NB], [1, AGW]],
                            ),
                        )

                    ps_ag = ps_agg.tile([O + 1, NL], f32, tag="agg")
                    bga = None
                    for jc in range(JC):
                        jg, jo = jc // NB, jc % NB
                        if no_bias_dma:
                            if bga is None:
                                bga = biasg.tile([P, NB, 512], f32, tag="biasg")
                                for ib in range(NB):
                                    nc.sync.dma_start(
                                        out=bga[:, ib, :],
                                        in_=bias_in[b, ib * P:(ib + 1) * P, 0:512],
                                    )
                        elif jo == 0:
                            # one 1MB DMA per key-group: [row block, 4 iblocks, 512]
                            bga = biasg.tile([P, NB, 512], f32, tag="biasg")
                            nc.sync.dma_start(
                                out=bga,
                                in_=bass.AP(
                                    tensor=bias_in.tensor,
                                    offset=b * NL * N + jg * 512,
                                    ap=[[N, P], [P * N, NB], [1, 512]],
                                ),
                            )
                        rank, nb_r = jc // NB, jc % NB
                        ps_bT = ps_big.tile([P, NL], f32, tag="biasT")
                        for ib in range(NB):
                            nc.tensor.transpose(
                                ps_bT[:, ib * P:(ib + 1) * P],
                                bga[:, ib, jo * P:(jo + 1) * P], ident,
                            )
                        f2c = sfall[:, rank, nb_r, O + 1:O + 2]
                        nf2c = sfall[:, rank, nb_r, O + 2:O + 3]
                        # softmax rows are shift-invariant: drop f1[i] from the
                        # logits (cancels in num/den). lrelu(v)+bias - f1 =
                        # f2[j] + bias_mat^T + (1-slope)*relu(-v).
                        w = work.tile([P, NL], f32, tag="w")
                        if jc % 16 < 9:
                            # DVE path: w = max(-0.99*(f1+f2), 0) + biasT
                            t = work.tile([P, NL], f32, tag="t")
                            nc.vector.tensor_scalar(
                                t, f1_reps[b], f2c, -(1.0 - SLOPE),
                                ALU.add, ALU.mult,
                            )
                            nc.vector.scalar_tensor_tensor(
                                w, t, 0.0, ps_bT, ALU.max, ALU.add
                            )
                        else:
                            r = work.tile([P, NL], f32, tag="r")
                            nc.scalar.activation(
                                r, f1_reps[b], AFT.Relu, bias=nf2c, scale=-1.0
                            )
                            nc.vector.scalar_tensor_tensor(
                                w, r, 1.0 - SLOPE, ps_bT, ALU.mult, ALU.add
                            )
                        e = work.tile([P, NL], f32, tag="e")
                        nc.scalar.activation(e, w, AFT.Exp, bias=f2c)
                        nc.tensor.matmul(
                            ps_ag, lhsT=sfall[:, rank, nb_r, 0:O + 1], rhs=e,
                            start=(jc == 0), stop=(jc == JC - 1),
                        )

                    ps_ags.append(ps_ag)

                # batched tails: cluster Ln then Exp to minimize table loads
                lnds = []
                for b in range(B):
                    lnd = tailp.tile([1, NL], f32, tag="lnd")
                    nc.scalar.activation(lnd, ps_ags[b][O:O + 1, :], AFT.Ln)
                    lnds.append(lnd)
                for b in range(B):
                    rrow = tailp.tile([1, NL], f32, tag="rrow")
                    nc.scalar.activation(rrow, lnds[b], AFT.Exp, scale=-1.0)
                    ps_bc = ps_misc.tile([O, NL], f32, tag="pmisc")
                    nc.tensor.matmul(ps_bc, lhsT=ones_o, rhs=rrow, start=True, stop=True)
                    nums = tailp.tile([O, NL], f32, tag="nums")
                    nc.vector.tensor_copy(nums, ps_ags[b][0:O, :])
                    nc.vector.tensor_tensor(
                        valsT[:, b * NL:(b + 1) * NL], nums, ps_bc, ALU.mult
                    )

                # ---------- BatchNorm stats + AllReduce ----------
                ssum = tailp.tile([O, 1], f32, tag="ssum")
                nc.vector.tensor_reduce(ssum, valsT, axis=mybir.AxisListType.X, op=ALU.add)
                sqt = persist.tile([O, B * NL], f32, tag="sqt")
                nc.scalar.activation(sqt, valsT, AFT.Square)
                ssq = tailp.tile([O, 1], f32, tag="ssq")
                nc.vector.tensor_reduce(ssq, sqt, axis=mybir.AxisListType.X, op=ALU.add)
                stt = tailp.tile([O, 2], f32, tag="stt")
                nc.vector.tensor_copy(stt[:, 0:1], ssum)
                nc.vector.tensor_copy(stt[:, 1:2], ssq)
                nc.sync.dma_start(out=st_in.ap(), in_=stt)
                if not no_cc:
                    nc.gpsimd.collective_compute(
                        "AllGather", ALU.bypass, replica_groups=rg,
                        ins=[st_in.ap()], outs=[st_out.ap()],
                    )
                # gather per-rank partials [o, (sum,sumsq), rank] and reduce
                tot3 = tailp.tile([O, 2, R], f32, tag="tot3")
                nc.sync.dma_start(
                    out=tot3,
                    in_=bass.AP(
                        tensor=st_out.ap().tensor, offset=0,
                        ap=[[2, O], [1, 2], [2 * O, R]],
                    ),
                )
                tot = tailp.tile([O, 2], f32, tag="tot")
                nc.vector.tensor_reduce(
                    tot, tot3, axis=mybir.AxisListType.X, op=ALU.add
                )

                mean = tailp.tile([O, 1], f32, tag="mean")
                nc.vector.tensor_scalar_mul(mean, tot[:, 0:1], 1.0 / (B * N))
                ex2 = tailp.tile([O, 1], f32, tag="ex2")
                nc.vector.tensor_scalar_mul(ex2, tot[:, 1:2], 1.0 / (B * N))
                msq = tailp.tile([O, 1], f32, tag="msq")
                nc.scalar.activation(msq, mean, AFT.Square)
                var = tailp.tile([O, 1], f32, tag="var")
                nc.vector.tensor_tensor(var, ex2, msq, ALU.subtract)
                lnv = tailp.tile([O, 1], f32, tag="lnv")
                nc.scalar.activation(lnv, var, AFT.Ln, bias=eps_t)
                istd = tailp.tile([O, 1], f32, tag="istd")
                nc.scalar.activation(istd, lnv, AFT.Exp, scale=-0.5)
                scal = tailp.tile([O, 1], f32, tag="scal")
                nc.vector.tensor_tensor(scal, istd, gam, ALU.mult)
                mscal = tailp.tile([O, 1], f32, tag="mscal")
                nc.vector.tensor_tensor(mscal, mean, scal, ALU.mult)
                shift = tailp.tile([O, 1], f32, tag="shift")
                nc.vector.tensor_tensor(shift, bet, mscal, ALU.subtract)

                ret = persist.tile([O, B * NL], f32, tag="ret")
                nc.scalar.activation(ret, valsT, AFT.Identity, bias=shift, scale=scal)
                pos = persist.tile([O, B * NL], f32, tag="pos")
                nc.scalar.activation(pos, ret, AFT.Relu)
                mng = persist.tile([O, B * NL], f32, tag="mng")
                nc.vector.tensor_scalar_min(mng, ret, 0.0)
                em = persist.tile([O, B * NL], f32, tag="em")
                nc.scalar.activation(em, mng, AFT.Exp)
                fin = persist.tile([O, B * NL], f32, tag="fin")
                nc.vector.scalar_tensor_tensor(fin, pos, -1.0, em, ALU.add, ALU.add)

                # ---------- output transpose + store ----------
                for b in range(B):
                    for nb in range(NB):
                        c0 = b * NL + nb * P
                        ps_oT = ps_misc.tile([P, O], f32, tag="pmisc")
                        nc.tensor.transpose(ps_oT, fin[:, c0:c0 + P], ident[0:O, 0:O])
                        oT = stage.tile([P, O], f32, tag="oT")
                        nc.vector.tensor_copy(oT, ps_oT)
                        nc.sync.dma_start(
                            out=out_ext[b, nb * P:(nb + 1) * P, :], in_=oT
                        )


            def _dma_body():
                zt = None
                for b in range(B):
                    for jg in range(JG):
                        bga = biasg.tile([P, NB, 512], f32, tag="biasg")
                        dma_engs = [nc.sync, nc.scalar, nc.sync, nc.scalar]
                        if dma8:
                            for ib in range(NB):
                                for hh in range(2):
                                    dma_engs[(2 * ib + hh) % 4].dma_start(
                                        out=bga[:, ib, hh * 256:(hh + 1) * 256],
                                        in_=bias_in[b, ib * P:(ib + 1) * P,
                                                    jg * 512 + hh * 256:
                                                    jg * 512 + (hh + 1) * 256],
                                    )
                        else:
                            for ib in range(NB):
                                dma_engs[ib].dma_start(
                                    out=bga[:, ib, :],
                                    in_=bias_in[b, ib * P:(ib + 1) * P,
                                                jg * 512:(jg + 1) * 512],
                                )
                    for nb in range(NB):
                        seq_t = stage.tile([P, F], f32, tag="seq_t")
                        nc.sync.dma_start(
                            out=seq_t, in_=seq_in[b, nb * P:(nb + 1) * P, :]
                        )
                for b in range(B):
                    sfall = perb.tile([P, R, NB, AGW], f32, tag="sfall")
                    for rank in range(R):
                        nc.sync.dma_start(
                            out=sfall[:, rank, :, :],
                            in_=bass.AP(
                                tensor=ag_out.ap().tensor,
                                offset=(rank * B * NL + b * NL) * AGW,
                                ap=[[AGW, P], [P * AGW, NB], [1, AGW]],
                            ),
                        )
                zt = stage.tile([P, O], f32, tag="oT")
                nc.vector.memset(zt, 0.0)
                for b in range(B):
                    for nb in range(NB):
                        nc.sync.dma_start(
                            out=out_ext[b, nb * P:(nb + 1) * P, :], in_=zt
                        )

            for _rep in range(n_reps):
                if dma_only:
                    _dma_body()
                else:
                    _rep_body()

    nc.compile()
    _CACHE[key] = nc
    return nc


def kernel(seq, bias_mat, W1, w2, b2, w3, b3, gamma, beta):
    seq = np.ascontiguousarray(seq, dtype=np.float32)
    bias_mat = np.ascontiguousarray(bias_mat, dtype=np.float32)
    w1t = np.ascontiguousarray(np.asarray(W1, dtype=np.float32).T)
    w2c = np.asarray(w2, dtype=np.float32).reshape(O, 1)
    w3c = np.asarray(w3, dtype=np.float32).reshape(O, 1)
    gam = np.asarray(gamma, dtype=np.float32).reshape(O, 1)
    bet = np.asarray(beta, dtype=np.float32).reshape(O, 1)
    sc = np.array([[float(b2), float(b3)]], dtype=np.float32)

    nc = _build_program(f32r_agg=True, dma8=True)

    in_maps = []
    for c in range(R):
        in_maps.append({
            "seq_loc": np.ascontiguousarray(seq[:, c * NL:(c + 1) * NL, :]),
            "bias_loc": np.ascontiguousarray(bias_mat[:, c * NL:(c + 1) * NL, :]),
            "w1t": w1t,
            "w2c": w2c,
            "w3c": w3c,
            "gamma_c": gam,
            "beta_c": bet,
            "scalars": sc,
        })

    res = run_bass_kernel_spmd(nc, in_maps, core_ids=list(range(R)))
    out = np.concatenate([res.results[c]["out_loc"] for c in range(R)], axis=1)
    return out



# revision 6
# speedup vs baseline: 1.7806x; 1.7806x over previous
"""GAT attention head (nn_AttnHead) on 8 Trainium2 NeuronCores.

Strategy (row-sharded, per sharding hint):
  - Core c owns query rows i in [c*512, (c+1)*512) for both batches.
  - Each core projects its own seq slice -> seq_fts (64ch), computes f1
    locally; seq_fts^T, ones, f2 are AllGathered so every core has all keys.
  - Attention runs in TRANSPOSED layout [j (keys, partitions), i (queries,
    free)]. The bias matrix is pre-transposed per core ON HOST to
    [B, N, NL] (key-major), pre-scaled by 1/(1-slope), and converted to
    bf16 — so the N x N PE transposes of the baseline disappear entirely
    and bias HBM traffic halves.
  - Per (b, j-chunk) tile: t2 = max(-f1[i]-f2[j], 0)   (DVE ts, bf16 2x)
                           w  = t2 + biasT'            (DVE tt, bf16 2x)
                           e  = exp(0.99*w + f2[j])    (ACT, fp32 out)
    using lrelu(v) = v + (1-s)*relu(-v) and softmax shift-invariance to
    drop f1 from the logits; the (1-s) factor rides the ACT scale and the
    host pre-scales bias by 1/(1-s) to compensate.
  - One PE matmul per (b, j-chunk) with lhsT = [seq_fts | ones] (f32r)
    accumulates numerator AND softmax denominator in PSUM.
  - BatchNorm batch stats via a tiny AllGather; normalize + ELU on chip;
    PE-transpose the [64, i] result back to [i, 64] rows and DMA out.
"""

import numpy as np
import ml_dtypes

import concourse.bass as bass
import concourse.bacc as bacc
import concourse.tile as tile
from concourse import mybir
from concourse.bass_utils import run_bass_kernel_spmd

B, N, F, O = 2, 4096, 256, 64
P = 128
R = 8                 # cores
NL = N // R           # 512 local query rows per core
NB = NL // P          # 4 row blocks
JC = N // P           # 32 key chunks of 128
JG = JC // NB         # 8 key groups of 512
AGW = O + 3           # AllGather payload: [sfT(64) | ones | f2 | -f2]
SLOPE = 0.01
EPS = 1e-5
f32 = mybir.dt.float32
f32r = mybir.dt.float32r
bf16 = mybir.dt.bfloat16
AFT = mybir.ActivationFunctionType
ALU = mybir.AluOpType

_CACHE = {}


def _build_program(n_reps=1, dma_only=False, no_cc=False, bias_alt=True,
                   pool_w=0, f32r_agg=True, dma8=True):
    # f32r_agg/dma8 kept for test.py signature compat (always-on behavior)
    key = (n_reps, dma_only, no_cc, bias_alt, pool_w)
    if key in _CACHE:
        return _CACHE[key]

    nc = bacc.Bacc("TRN2", target_bir_lowering=False, debug=False, num_devices=R)

    seq_in = nc.dram_tensor("seq_loc", [B, NL, F], f32, kind="ExternalInput").ap()
    biasT_in = nc.dram_tensor("biasT_loc", [B, N, NL], bf16, kind="ExternalInput").ap()
    w1t_in = nc.dram_tensor("w1t", [F, O], f32, kind="ExternalInput").ap()
    w2_in = nc.dram_tensor("w2c", [O, 1], f32, kind="ExternalInput").ap()
    w3_in = nc.dram_tensor("w3c", [O, 1], f32, kind="ExternalInput").ap()
    gam_in = nc.dram_tensor("gamma_c", [O, 1], f32, kind="ExternalInput").ap()
    bet_in = nc.dram_tensor("beta_c", [O, 1], f32, kind="ExternalInput").ap()
    sc_in = nc.dram_tensor("scalars", [1, 2], f32, kind="ExternalInput").ap()
    out_ext = nc.dram_tensor("out_loc", [B, NL, O], f32, kind="ExternalOutput").ap()

    ag_in = nc.dram_tensor("ag_in", [B * NL, AGW], f32)
    ag_out = nc.dram_tensor("ag_out", [R * B * NL, AGW], f32, addr_space="Shared")
    st_in = nc.dram_tensor("st_in", [O, 2], f32)
    st_out = nc.dram_tensor("st_out", [R * O, 2], f32, addr_space="Shared")

    ident_d = nc.inline_tensor(np.eye(P, dtype=np.float32), name="ident")
    rg = [list(range(R))]

    with tile.TileContext(nc, num_cores=R) as tc:
        with (
            tc.tile_pool(name="consts", bufs=1) as consts,
            tc.tile_pool(name="perb", bufs=2) as perb,
            tc.tile_pool(name="persist", bufs=1) as persist,
            tc.tile_pool(name="biasg", bufs=6) as biasg,
            tc.tile_pool(name="work", bufs=4) as work,
            tc.tile_pool(name="stage", bufs=8) as stage,
            tc.tile_pool(name="tailp", bufs=2) as tailp,
            tc.tile_pool(name="ps_agg", bufs=2, space="PSUM") as ps_agg,
            tc.tile_pool(name="ps_proj", bufs=1, space="PSUM") as ps_proj,
            tc.tile_pool(name="ps_misc", bufs=2, space="PSUM") as ps_misc,
        ):
            # ---------- constants ----------
            ident = consts.tile([P, P], f32)
            nc.sync.dma_start(out=ident, in_=ident_d.ap())
            w1a = consts.tile([P, O], f32)
            nc.sync.dma_start(out=w1a, in_=w1t_in[0:P, :])
            w1b = consts.tile([P, O], f32)
            nc.sync.dma_start(out=w1b, in_=w1t_in[P:F, :])
            w2c = consts.tile([O, 1], f32)
            nc.sync.dma_start(out=w2c, in_=w2_in)
            w3c = consts.tile([O, 1], f32)
            nc.sync.dma_start(out=w3c, in_=w3_in)
            gam = consts.tile([O, 1], f32)
            nc.sync.dma_start(out=gam, in_=gam_in)
            bet = consts.tile([O, 1], f32)
            nc.sync.dma_start(out=bet, in_=bet_in)
            b2t = consts.tile([1, 1], f32)
            nc.sync.dma_start(out=b2t, in_=sc_in[0:1, 0:1])
            b3r = consts.tile([P, 1], f32)
            nc.gpsimd.dma_start(
                out=b3r,
                in_=bass.AP(tensor=sc_in.tensor, offset=1, ap=[[0, P], [1, 1]]),
            )
            # constants built on ACT (so matmuls reading them wait on ACT only)
            ones_r = consts.tile([1, P], f32r)
            nc.scalar.activation(ones_r, ident[0:1, :], AFT.Copy, bias=1.0, scale=0.0)
            ones_o = consts.tile([1, O], f32r)
            nc.scalar.activation(ones_o, ident[0:1, 0:O], AFT.Copy, bias=1.0, scale=0.0)
            eps_t = consts.tile([O, 1], f32)
            nc.scalar.activation(eps_t, ident[0:O, 0:1], AFT.Copy, bias=EPS, scale=0.0)
            b3n = consts.tile([P, 1], f32)
            nc.vector.tensor_scalar_mul(b3n, b3r, -1.0)
            nb2t = consts.tile([1, 1], f32)
            nc.vector.tensor_scalar_mul(nb2t, b2t, -1.0)

            valsT = persist.tile([O, B * NL], f32, tag="valsT")

            def _rep_body():
                # ---------- phase A: projection + AllGather ----------
                stgall = persist.tile([P, B, NB, AGW], f32, tag="stgall")
                nf1s = []
                for b in range(B):
                    ps_sf = ps_proj.tile([O, NL], f32, tag="ps_sf")
                    for nb in range(NB):
                        seq_t = stage.tile([P, F], f32, tag="seq_t")
                        nc.sync.dma_start(
                            out=seq_t, in_=seq_in[b, nb * P:(nb + 1) * P, :]
                        )
                        ps_sT = ps_misc.tile([P, 2, P], f32, tag="pmisc")
                        nc.tensor.transpose(ps_sT[:, 0, :], seq_t[:, 0:P], ident)
                        nc.tensor.transpose(ps_sT[:, 1, :], seq_t[:, P:F], ident)
                        sT = stage.tile([P, 2, P], f32, tag="sT")
                        nc.vector.tensor_copy(sT, ps_sT)
                        nc.tensor.matmul(
                            ps_sf[:, nb * P:(nb + 1) * P], lhsT=w1a, rhs=sT[:, 0, :],
                            start=True, stop=False,
                        )
                        nc.tensor.matmul(
                            ps_sf[:, nb * P:(nb + 1) * P], lhsT=w1b, rhs=sT[:, 1, :],
                            start=False, stop=True,
                        )
                    sf_loc = perb.tile([O, NL], f32, tag="sf_loc")
                    nc.vector.tensor_copy(sf_loc, ps_sf)

                    # nf1row = -(w2 . sf) - b2  (negated f1; t2 wants -f1)
                    ps_f1 = ps_misc.tile([1, NL], f32, tag="pmisc")
                    nc.tensor.matmul(ps_f1, lhsT=w2c, rhs=sf_loc, start=True, stop=True)
                    nf1row = stage.tile([1, NL], f32r, tag="nf1row")
                    nc.scalar.activation(nf1row, ps_f1, AFT.Identity,
                                         bias=nb2t, scale=-1.0)
                    ps_rep = ps_misc.tile([P, NL], f32, tag="pmisc")
                    nc.tensor.matmul(ps_rep, lhsT=ones_r,
                                     rhs=nf1row, start=True, stop=True)
                    nf1 = perb.tile([P, NL], bf16, tag="nf1")
                    nc.vector.tensor_copy(nf1, ps_rep)
                    nf1s.append(nf1)

                    for nb in range(NB):
                        ps_sfT = ps_misc.tile([P, O], f32, tag="pmisc")
                        nc.tensor.transpose(
                            ps_sfT, sf_loc[:, nb * P:(nb + 1) * P], ident[0:O, 0:O]
                        )
                        ps_f2T = ps_misc.tile([P, 1], f32, tag="pmisc")
                        nc.tensor.matmul(
                            ps_f2T, lhsT=sf_loc[:, nb * P:(nb + 1) * P], rhs=w3c,
                            start=True, stop=True,
                        )
                        nc.vector.tensor_copy(stgall[:, b, nb, 0:O], ps_sfT)
                        nc.scalar.activation(
                            stgall[:, b, nb, O:O + 1], ident[:, 0:1],
                            AFT.Copy, bias=1.0, scale=0.0,
                        )
                        nc.scalar.activation(
                            stgall[:, b, nb, O + 1:O + 2], ps_f2T, AFT.Identity,
                            bias=b3r,
                        )
                        nc.scalar.activation(
                            stgall[:, b, nb, O + 2:O + 3], ps_f2T, AFT.Identity,
                            bias=b3n, scale=-1.0,
                        )
                # one DMA -> ag_in (single queue sem for the collective to wait on)
                nc.sync.dma_start(
                    out=bass.AP(
                        tensor=ag_in.ap().tensor, offset=0,
                        ap=[[AGW, P], [NL * AGW, B], [P * AGW, NB], [1, AGW]],
                    ),
                    in_=stgall,
                )
                if not no_cc:
                    nc.gpsimd.collective_compute(
                        "AllGather", ALU.bypass, replica_groups=rg,
                        ins=[ag_in.ap()], outs=[ag_out.ap()],
                    )

                # ---------- phase B: attention main loop (batch-interleaved) ----
                sfalls, ps_ags = [], []
                for b in range(B):
                    sfall = perb.tile([P, R, NB, AGW], f32, tag="sfall")
                    for rank in range(R):
                        nc.sync.dma_start(
                            out=sfall[:, rank, :, :],
                            in_=bass.AP(
                                tensor=ag_out.ap().tensor,
                                offset=(rank * B * NL + b * NL) * AGW,
                                ap=[[AGW, P], [P * AGW, NB], [1, AGW]],
                            ),
                        )
                    sfalls.append(sfall)
                    ps_ag_i = ps_agg.tile([O + 1, NL], f32, tag="agg")
                    ps_ags.append(ps_ag_i)

                bgas = [None, None]
                dma_engs = [nc.sync, nc.scalar] if bias_alt else [nc.sync, nc.sync]
                for jc in range(JC):
                    jg, jo = jc // NB, jc % NB
                    for b in range(B):
                        if jo == 0:
                            bga = biasg.tile([P, NB, NL], bf16, tag="biasg")
                            for ib in range(NB):
                                dma_engs[(2 * b + ib) % 2].dma_start(
                                    out=bga[:, ib, :],
                                    in_=biasT_in[
                                        b, (jg * NB + ib) * P:(jg * NB + ib + 1) * P, :
                                    ],
                                )
                            bgas[b] = bga
                        f2c = sfalls[b][:, jg, jo, O + 1:O + 2]
                        nf2c = sfalls[b][:, jg, jo, O + 2:O + 3]
                        t2 = work.tile([P, NL], bf16, tag="t2")
                        nc.vector.tensor_scalar(
                            t2, nf1s[b], nf2c, 0.0, ALU.add, ALU.max
                        )
                        w = work.tile([P, NL], bf16, tag="w")
                        weng = nc.gpsimd if (jc % JC) < pool_w else nc.vector
                        weng.tensor_tensor(w, t2, bgas[b][:, jo, :], ALU.add)
                        e = work.tile([P, NL], f32r, tag="e")
                        nc.scalar.activation(e, w, AFT.Exp, bias=f2c,
                                             scale=(1.0 - SLOPE))
                        nc.tensor.matmul(
                            ps_ags[b],
                            lhsT=sfalls[b][:, jg, jo, 0:O + 1].bitcast(f32r),
                            rhs=e,
                            start=(jc == 0), stop=(jc == JC - 1),
                        )

                # batched tails: cluster Ln then Exp to minimize table loads
                lnds = []
                for b in range(B):
                    lnd = tailp.tile([1, NL], f32, tag="lnd")
                    nc.scalar.activation(lnd, ps_ags[b][O:O + 1, :], AFT.Ln)
                    lnds.append(lnd)
                ssums = []
                for b in range(B):
                    rrow = tailp.tile([1, NL], f32r, tag="rrow")
                    nc.scalar.activation(rrow, lnds[b], AFT.Exp, scale=-1.0)
                    ps_bc = ps_misc.tile([O, NL], f32, tag="pmisc")
                    nc.tensor.matmul(ps_bc, lhsT=ones_o,
                                     rhs=rrow, start=True, stop=True)
                    bc = tailp.tile([O, NL], f32, tag="bc")
                    nc.vector.tensor_copy(bc, ps_bc)
                    ssum_b = tailp.tile([O, 1], f32, tag="ssum_b")
                    nc.vector.scalar_tensor_tensor(
                        valsT[:, b * NL:(b + 1) * NL], ps_ags[b][0:O, :], 0.0,
                        bc, ALU.add, ALU.mult, accum_out=ssum_b,
                    )
                    ssums.append(ssum_b)

                # ---------- BatchNorm stats + AllReduce ----------
                ssum = tailp.tile([O, 1], f32, tag="ssum")
                nc.vector.tensor_tensor(ssum, ssums[0], ssums[1], ALU.add)
                sqt = persist.tile([O, B * NL], f32, tag="sqt")
                nc.gpsimd.tensor_tensor(sqt, valsT, valsT, ALU.mult)
                ssq = tailp.tile([O, 1], f32, tag="ssq")
                nc.vector.tensor_reduce(ssq, sqt, axis=mybir.AxisListType.X, op=ALU.add)
                stt = tailp.tile([O, 2], f32, tag="stt")
                nc.vector.tensor_copy(stt[:, 0:1], ssum)
                nc.vector.tensor_copy(stt[:, 1:2], ssq)
                nc.sync.dma_start(out=st_in.ap(), in_=stt)
                if not no_cc:
                    nc.gpsimd.collective_compute(
                        "AllGather", ALU.bypass, replica_groups=rg,
                        ins=[st_in.ap()], outs=[st_out.ap()],
                    )
                # gather per-rank partials [o, (sum,sumsq), rank] and reduce
                tot3 = tailp.tile([O, 2, R], f32, tag="tot3")
                nc.sync.dma_start(
                    out=tot3,
                    in_=bass.AP(
                        tensor=st_out.ap().tensor, offset=0,
                        ap=[[2, O], [1, 2], [2 * O, R]],
                    ),
                )
                tot = tailp.tile([O, 2], f32, tag="tot")
                nc.vector.tensor_reduce(
                    tot, tot3, axis=mybir.AxisListType.X, op=ALU.add
                )

                mean = tailp.tile([O, 1], f32, tag="mean")
                nc.vector.tensor_scalar_mul(mean, tot[:, 0:1], 1.0 / (B * N))
                ex2 = tailp.tile([O, 1], f32, tag="ex2")
                nc.vector.tensor_scalar_mul(ex2, tot[:, 1:2], 1.0 / (B * N))
                msq = tailp.tile([O, 1], f32, tag="msq")
                nc.scalar.activation(msq, mean, AFT.Square)
                var = tailp.tile([O, 1], f32, tag="var")
                nc.vector.tensor_tensor(var, ex2, msq, ALU.subtract)
                lnv = tailp.tile([O, 1], f32, tag="lnv")
                nc.scalar.activation(lnv, var, AFT.Ln, bias=eps_t)
                istd = tailp.tile([O, 1], f32, tag="istd")
                nc.scalar.activation(istd, lnv, AFT.Exp, scale=-0.5)
                scal = tailp.tile([O, 1], f32, tag="scal")
                nc.vector.tensor_tensor(scal, istd, gam, ALU.mult)
                mscal = tailp.tile([O, 1], f32, tag="mscal")
                nc.vector.tensor_tensor(mscal, mean, scal, ALU.mult)
                shift = tailp.tile([O, 1], f32, tag="shift")
                nc.vector.tensor_tensor(shift, bet, mscal, ALU.subtract)

                ret = persist.tile([O, B * NL], f32, tag="ret")
                nc.scalar.activation(ret, valsT, AFT.Identity, bias=shift, scale=scal)
                pos = persist.tile([O, B * NL], f32, tag="pos")
                nc.scalar.activation(pos, ret, AFT.Relu)
                mng = persist.tile([O, B * NL], f32, tag="mng")
                nc.vector.tensor_scalar_min(mng, ret, 0.0)
                em = persist.tile([O, B * NL], f32, tag="em")
                nc.scalar.activation(em, mng, AFT.Exp)
                fin = persist.tile([O, B * NL], f32, tag="fin")
                nc.vector.scalar_tensor_tensor(fin, pos, -1.0, em, ALU.add, ALU.add)

                # ---------- output transpose + store ----------
                for b in range(B):
                    for nb in range(NB):
                        c0 = b * NL + nb * P
                        ps_oT = ps_misc.tile([P, O], f32, tag="pmisc")
                        nc.tensor.transpose(ps_oT, fin[:, c0:c0 + P], ident[0:O, 0:O])
                        oT = stage.tile([P, O], f32, tag="oT")
                        nc.vector.tensor_copy(oT, ps_oT)
                        nc.sync.dma_start(
                            out=out_ext[b, nb * P:(nb + 1) * P, :], in_=oT
                        )

            def _dma_body():
                for b in range(B):
                    for jg in range(JG):
                        bga = biasg.tile([P, NB, NL], bf16, tag="biasg")
                        dma_engs = [nc.sync, nc.scalar] if bias_alt \
                            else [nc.sync, nc.sync]
                        for ib in range(NB):
                            dma_engs[(2 * b + ib) % 2].dma_start(
                                out=bga[:, ib, :],
                                in_=biasT_in[
                                    b, (jg * NB + ib) * P:(jg * NB + ib + 1) * P, :
                                ],
                            )
                    for nb in range(NB):
                        seq_t = stage.tile([P, F], f32, tag="seq_t")
                        nc.sync.dma_start(
                            out=seq_t, in_=seq_in[b, nb * P:(nb + 1) * P, :]
                        )
                for b in range(B):
                    sfall = perb.tile([P, R, NB, AGW], f32, tag="sfall")
                    for rank in range(R):
                        nc.sync.dma_start(
                            out=sfall[:, rank, :, :],
                            in_=bass.AP(
                                tensor=ag_out.ap().tensor,
                                offset=(rank * B * NL + b * NL) * AGW,
                                ap=[[AGW, P], [P * AGW, NB], [1, AGW]],
                            ),
                        )
                zt = stage.tile([P, O], f32, tag="oT")
                nc.vector.memset(zt, 0.0)
                for b in range(B):
                    for nb in range(NB):
                        nc.sync.dma_start(
                            out=out_ext[b, nb * P:(nb + 1) * P, :], in_=zt
                        )

            for _rep in range(n_reps):
                if dma_only:
                    _dma_body()
                else:
                    _rep_body()

    nc.compile()
    _CACHE[key] = nc
    return nc


def _prep_inputs(seq, bias_mat, W1, w2, b2, w3, b3, gamma, beta):
    seq = np.ascontiguousarray(seq, dtype=np.float32)
    bias_mat = np.asarray(bias_mat, dtype=np.float32)
    w1t = np.ascontiguousarray(np.asarray(W1, dtype=np.float32).T)
    w2c = np.asarray(w2, dtype=np.float32).reshape(O, 1)
    w3c = np.asarray(w3, dtype=np.float32).reshape(O, 1)
    gam = np.asarray(gamma, dtype=np.float32).reshape(O, 1)
    bet = np.asarray(beta, dtype=np.float32).reshape(O, 1)
    sc = np.array([[float(b2), float(b3)]], dtype=np.float32)

    in_maps = []
    for c in range(R):
        # host-side: transpose core's row-block to key-major [B, N, NL],
        # pre-scale by 1/(1-slope) (the ACT exp applies *(1-slope) on chip),
        # and convert to bf16.
        bT = bias_mat[:, c * NL:(c + 1) * NL, :].transpose(0, 2, 1)
        bT = np.ascontiguousarray(bT) * (1.0 / (1.0 - SLOPE))
        bT = bT.astype(ml_dtypes.bfloat16)
        in_maps.append({
            "seq_loc": np.ascontiguousarray(seq[:, c * NL:(c + 1) * NL, :]),
            "biasT_loc": bT,
            "w1t": w1t,
            "w2c": w2c,
            "w3c": w3c,
            "gamma_c": gam,
            "beta_c": bet,
            "scalars": sc,
        })
    return in_maps


def kernel(seq, bias_mat, W1, w2, b2, w3, b3, gamma, beta):
    in_maps = _prep_inputs(seq, bias_mat, W1, w2, b2, w3, b3, gamma, beta)
    nc = _build_program()
    res = run_bass_kernel_spmd(nc, in_maps, core_ids=list(range(R)))
    out = np.concatenate([res.results[c]["out_loc"] for c in range(R)], axis=1)
    return out


# revision 25
# speedup vs baseline: 2.9150x; 1.6371x over previous
"""GAT attention head (nn_AttnHead) on 8 Trainium2 NeuronCores.

Strategy (row-sharded, per sharding hint):
  - Core c owns query rows i in [c*512, (c+1)*512) for both batches.
  - Each core projects its own seq slice -> seq_fts (64ch, fp16 inputs via
    XBAR DMA-transpose), computes f1 locally; seq_fts^T, ones, f2 are
    AllGathered (fp16 payload) so every core has all keys.
  - Attention runs in TRANSPOSED layout [j (keys, partitions), i (queries,
    free)].  The bias matrix is pre-transposed per core ON HOST to
    [B, N, NL] (key-major), pre-scaled by 1/(1-slope), and converted to
    fp16 -- no N x N PE transposes on chip, and bias HBM traffic halves.
  - Per (b, j-chunk) tile: t2 = max(-f1[i]-f2[j], 0)   (DVE ts, fp16 2x)
                           w  = t2 + biasT'            (DVE tt, fp16 2x)
                           e  = exp(0.99*w + f2[j])    (ACT, fp16 out)
    using lrelu(v) = v + (1-s)*relu(-v) and softmax shift-invariance to
    drop f1 from the logits; the (1-s) factor rides the ACT scale and the
    host pre-scales bias by 1/(1-s) to compensate.
  - One fp16 PE matmul per (b, j-chunk) with lhsT = [seq_fts | ones]
    accumulates numerator AND softmax denominator in PSUM (fp32).
  - All bias DMAs issue from the sync (SP) queue only -- issuing from the
    scalar queue stalls ACT exp.  Activation table set 6 (ln+exp) is
    preloaded manually so the greedy per-func inserter never thrashes.
  - Softmax reciprocal on DVE (InstReciprocal), broadcast via f32r rank-1
    matmul; BN sum/sumsq fused into the normalize stt ops via accum_out.
  - BatchNorm batch stats via a tiny AllGather; normalize + ELU on chip;
    PE-transpose the [64, i] result back to [i, 64] rows and DMA out.
"""

import numpy as np
import ml_dtypes

import concourse.bass as bass
import concourse.bacc as bacc
import concourse.tile as tile
from concourse import mybir
from concourse.bass_utils import run_bass_kernel_spmd

B, N, F, O = 2, 4096, 256, 64
P = 128
R = 8                 # cores
NL = N // R           # 512 local query rows per core
NB = NL // P          # 4 row blocks
JC = N // P           # 32 key chunks of 128
JG = JC // NB         # 8 key groups of 512
AGW = O + 3           # AllGather payload: [sfT(64) | ones | f2 | -f2]
SLOPE = 0.01
EPS = 1e-5
f32 = mybir.dt.float32
f32r = mybir.dt.float32r
bf16 = mybir.dt.bfloat16
fp16 = mybir.dt.float16
AFT = mybir.ActivationFunctionType
ALU = mybir.AluOpType

_CACHE = {}


def _build_program(n_reps=1, dma_only=False, no_cc=False, bias_alt=False,
                   pool_w=0, bias_gran=4, bf16_proj=True, no_cc_main=False,
                   no_cc_stats=False, deep=False, cc_sync=False,
                   local_rank=False, f32r_agg=True, dma8=True):
    # f32r_agg/dma8 kept for test.py signature compat (always-on behavior)
    if no_cc:
        no_cc_main = no_cc_stats = True
    key = (n_reps, dma_only, no_cc_main, no_cc_stats, bias_alt, pool_w,
           bias_gran, bf16_proj, deep, cc_sync, local_rank)
    if key in _CACHE:
        return _CACHE[key]

    nc = bacc.Bacc("TRN2", target_bir_lowering=False, debug=False, num_devices=R)

    seq_dt = fp16 if bf16_proj else f32
    seq_in = nc.dram_tensor("seq_loc", [B, NL, F], seq_dt, kind="ExternalInput").ap()
    biasT_in = nc.dram_tensor("biasT_loc", [B, N, NL], fp16, kind="ExternalInput").ap()
    w1t_in = nc.dram_tensor("w1t", [F, O], seq_dt, kind="ExternalInput").ap()
    w2_in = nc.dram_tensor("w2c", [O, 1], f32, kind="ExternalInput").ap()
    w3_in = nc.dram_tensor("w3c", [O, 1], f32, kind="ExternalInput").ap()
    gam_in = nc.dram_tensor("gamma_c", [O, 1], f32, kind="ExternalInput").ap()
    bet_in = nc.dram_tensor("beta_c", [O, 1], f32, kind="ExternalInput").ap()
    sc_in = nc.dram_tensor("scalars", [1, 2], f32, kind="ExternalInput").ap()
    out_ext = nc.dram_tensor("out_loc", [B, NL, O], f32, kind="ExternalOutput").ap()

    ag_in = nc.dram_tensor("ag_in", [B * NL, AGW], fp16)
    ag_out = nc.dram_tensor("ag_out", [R * B * NL, AGW], fp16, addr_space="Shared")
    st_in = nc.dram_tensor("st_in", [O, 2], f32)
    st_out = nc.dram_tensor("st_out", [R * O, 2], f32, addr_space="Shared")

    ident_d = nc.inline_tensor(np.eye(P, dtype=np.float32), name="ident")
    rg = [list(range(R))]

    with tile.TileContext(nc, num_cores=R) as tc:
        with (
            tc.tile_pool(name="consts", bufs=1) as consts,
            tc.tile_pool(name="perb", bufs=4 if deep else 2) as perb,
            tc.tile_pool(name="persist", bufs=2 if deep else 1) as persist,
            tc.tile_pool(name="biasg", bufs=6) as biasg,
            tc.tile_pool(name="work", bufs=6) as work,
            tc.tile_pool(name="stage", bufs=8) as stage,
            tc.tile_pool(name="tailp", bufs=4 if deep else 2) as tailp,
            tc.tile_pool(name="ps_agg", bufs=4 if deep else 2, space="PSUM") as ps_agg,
            tc.tile_pool(name="ps_proj", bufs=2 if deep else 1, space="PSUM") as ps_proj,
            tc.tile_pool(name="ps_misc", bufs=2, space="PSUM") as ps_misc,
        ):
            # ---------- constants ----------
            # preload the ln+exp activation table set so the greedy
            # per-func table inserter never thrashes between the exp-only
            # and ln-only sets (natural_log_exp_and_others = set 6)
            nc.scalar.add_instruction(mybir.InstLoadActFuncSet(
                name=nc.get_next_instruction_name(), ins=[], outs=[],
                act_func_set_id=6))
            ident = consts.tile([P, P], f32)
            nc.sync.dma_start(out=ident, in_=ident_d.ap())
            w1a = consts.tile([P, O], seq_dt)
            nc.sync.dma_start(out=w1a, in_=w1t_in[0:P, :])
            w1b = consts.tile([P, O], seq_dt)
            nc.sync.dma_start(out=w1b, in_=w1t_in[P:F, :])
            w2c = consts.tile([O, 1], f32)
            nc.sync.dma_start(out=w2c, in_=w2_in)
            w3c = consts.tile([O, 1], f32)
            nc.sync.dma_start(out=w3c, in_=w3_in)
            gam = consts.tile([O, 1], f32)
            nc.sync.dma_start(out=gam, in_=gam_in)
            bet = consts.tile([O, 1], f32)
            nc.sync.dma_start(out=bet, in_=bet_in)
            b2t = consts.tile([1, 1], f32)
            nc.sync.dma_start(out=b2t, in_=sc_in[0:1, 0:1])
            b3r = consts.tile([P, 1], f32)
            nc.gpsimd.dma_start(
                out=b3r,
                in_=bass.AP(tensor=sc_in.tensor, offset=1, ap=[[0, P], [1, 1]]),
            )
            # constants built on ACT (so matmuls reading them wait on ACT only)
            ones_r = consts.tile([1, P], f32r)
            nc.scalar.activation(ones_r, ident[0:1, :], AFT.Copy, bias=1.0, scale=0.0)
            ones_o = consts.tile([1, O], f32r)
            nc.scalar.activation(ones_o, ident[0:1, 0:O], AFT.Copy, bias=1.0, scale=0.0)
            eps_t = consts.tile([O, 1], f32)
            nc.scalar.activation(eps_t, ident[0:O, 0:1], AFT.Copy, bias=EPS, scale=0.0)
            b3n = consts.tile([P, 1], f32)
            nc.vector.tensor_scalar_mul(b3n, b3r, -1.0)
            nb2t = consts.tile([1, 1], f32)
            nc.vector.tensor_scalar_mul(nb2t, b2t, -1.0)

            valsT = persist.tile([O, B * NL], f32, tag="valsT")

            def _rep_body():
                # ---------- phase A: projection + AllGather ----------
                stgall = persist.tile([P, B, NB, AGW], fp16, tag="stgall")
                nc.vector.memset(stgall[:, :, :, O:O + 1], 1.0)
                f2locs, nf2locs = [], []
                if local_rank:
                    for b in range(B):
                        f2l = perb.tile([P, NB, 1], f32, tag="f2l")
                        f2locs.append(f2l)
                        nf2l = perb.tile([P, NB, 1], f32, tag="nf2l")
                        nf2locs.append(nf2l)
                nf1s = []
                for b in range(B):
                    ps_sf = ps_proj.tile([O, NL], f32, tag="ps_sf")
                    if bf16_proj:
                        sT = stage.tile([P, 2, NL], fp16, tag="sT")
                        for h in range(2):
                            nc.sync.dma_start(
                                out=sT[:, h, :],
                                in_=seq_in[b, :, h * P:(h + 1) * P],
                                transpose=True,
                            )
                        nc.tensor.matmul(ps_sf, lhsT=w1a, rhs=sT[:, 0, :],
                                         start=True, stop=False)
                        nc.tensor.matmul(ps_sf, lhsT=w1b, rhs=sT[:, 1, :],
                                         start=False, stop=True)
                    else:
                      for nb in range(NB):
                        seq_t = stage.tile([P, F], f32, tag="seq_t")
                        nc.sync.dma_start(
                            out=seq_t, in_=seq_in[b, nb * P:(nb + 1) * P, :]
                        )
                        ps_sT = ps_misc.tile([P, 2, P], f32, tag="pmisc")
                        nc.tensor.transpose(ps_sT[:, 0, :], seq_t[:, 0:P], ident)
                        nc.tensor.transpose(ps_sT[:, 1, :], seq_t[:, P:F], ident)
                        sT = stage.tile([P, 2, P], f32, tag="sT")
                        nc.vector.tensor_copy(sT, ps_sT)
                        nc.tensor.matmul(
                            ps_sf[:, nb * P:(nb + 1) * P], lhsT=w1a, rhs=sT[:, 0, :],
                            start=True, stop=False,
                        )
                        nc.tensor.matmul(
                            ps_sf[:, nb * P:(nb + 1) * P], lhsT=w1b, rhs=sT[:, 1, :],
                            start=False, stop=True,
                        )
                    sf_loc = perb.tile([O, NL], f32, tag="sf_loc")
                    nc.vector.tensor_copy(sf_loc, ps_sf)

                    # nf1row = -(w2 . sf) - b2  (negated f1; t2 wants -f1)
                    ps_f1 = ps_misc.tile([1, NL], f32, tag="pmisc")
                    nc.tensor.matmul(ps_f1, lhsT=w2c, rhs=sf_loc, start=True, stop=True)
                    nf1row = stage.tile([1, NL], f32r, tag="nf1row")
                    nc.vector.tensor_scalar(nf1row, ps_f1, -1.0, nb2t,
                                            ALU.mult, ALU.add)
                    ps_rep = ps_misc.tile([P, NL], f32, tag="pmisc")
                    nc.tensor.matmul(ps_rep, lhsT=ones_r,
                                     rhs=nf1row, start=True, stop=True)
                    nf1 = perb.tile([P, NL], fp16, tag="nf1")
                    nc.vector.tensor_copy(nf1, ps_rep)
                    nf1s.append(nf1)

                    for nb in range(NB):
                        ps_sfT = ps_misc.tile([P, O], f32, tag="pmisc")
                        nc.tensor.transpose(
                            ps_sfT, sf_loc[:, nb * P:(nb + 1) * P], ident[0:O, 0:O]
                        )
                        ps_f2T = ps_misc.tile([P, 1], f32, tag="pmisc")
                        nc.tensor.matmul(
                            ps_f2T, lhsT=sf_loc[:, nb * P:(nb + 1) * P], rhs=w3c,
                            start=True, stop=True,
                        )
                        nc.vector.tensor_copy(stgall[:, b, nb, 0:O], ps_sfT)
                        nc.vector.tensor_scalar(
                            stgall[:, b, nb, O + 1:O + 2], ps_f2T, 1.0, b3r,
                            ALU.mult, ALU.add,
                        )
                        nc.vector.tensor_scalar(
                            stgall[:, b, nb, O + 2:O + 3], ps_f2T, -1.0, b3n,
                            ALU.mult, ALU.add,
                        )
                        if local_rank:
                            nc.vector.tensor_scalar(
                                f2locs[b][:, nb, :], ps_f2T, 1.0, b3r,
                                ALU.mult, ALU.add,
                            )
                            nc.vector.tensor_scalar(
                                nf2locs[b][:, nb, :], ps_f2T, -1.0, b3n,
                                ALU.mult, ALU.add,
                            )
                # one DMA -> ag_in (single queue sem for the collective to wait on)
                nc.sync.dma_start(
                    out=bass.AP(
                        tensor=ag_in.ap().tensor, offset=0,
                        ap=[[AGW, P], [NL * AGW, B], [P * AGW, NB], [1, AGW]],
                    ),
                    in_=stgall,
                )
                if not no_cc_main:
                    nc.gpsimd.collective_compute(
                        "AllGather", ALU.bypass, replica_groups=rg,
                        ins=[ag_in.ap()], outs=[ag_out.ap()],
                    )

                # ---------- phase B: attention main loop (batch-interleaved) ----
                sfalls, ps_ags = [], []
                for b in range(B):
                    sfall = perb.tile([P, R, NB, AGW], fp16, tag="sfall")
                    for rank in range(R):
                        nc.sync.dma_start(
                            out=sfall[:, rank, :, :],
                            in_=bass.AP(
                                tensor=ag_out.ap().tensor,
                                offset=(rank * B * NL + b * NL) * AGW,
                                ap=[[AGW, P], [P * AGW, NB], [1, AGW]],
                            ),
                        )
                    sfalls.append(sfall)
                    ps_ag_i = ps_agg.tile([O + 1, NL], f32, tag="agg")
                    ps_ags.append(ps_ag_i)
                f2s, nf2s = [], []
                for b in range(B):
                    f2a = perb.tile([P, R, NB, 1], f32, tag="f2a")
                    nc.vector.tensor_copy(f2a, sfalls[b][:, :, :, O + 1:O + 2])
                    f2s.append(f2a)
                    nf2a = perb.tile([P, R, NB, 1], f32, tag="nf2a")
                    nc.vector.tensor_copy(nf2a, sfalls[b][:, :, :, O + 2:O + 3])
                    nf2s.append(nf2a)

                bgas = [None, None]
                dma_engs = [nc.sync, nc.scalar] if bias_alt else [nc.sync, nc.sync]
                # own-rank key chunks need no gather: schedule them first so
                # they overlap the AllGather.  partition_id isn't known at
                # trace time, so "own rank" here is a fixed chunk-reorder:
                # every core runs chunk order [own-ish first] -- we can't
                # know the rank statically, so instead keep natural order but
                # source rank-local data for ALL ranks from the gather, and
                # additionally allow chunk 'lr' tiles to source locally via
                # partition-id-independent approach: not possible statically.
                for jc in range(JC):
                    jg, jo = jc // NB, jc % NB
                    for b in range(B):
                        if jo == 0:
                            bga = biasg.tile([P, NB, NL], fp16, tag="biasg")
                            if bias_gran == 1:
                                dma_engs[b % 2].dma_start(
                                    out=bga,
                                    in_=bass.AP(
                                        tensor=biasT_in.tensor,
                                        offset=b * N * NL + jg * NB * P * NL,
                                        ap=[[NL, P], [P * NL, NB], [1, NL]],
                                    ),
                                )
                            else:
                                for ib in range(NB):
                                    dma_engs[(2 * b + ib) % 2].dma_start(
                                        out=bga[:, ib, :],
                                        in_=biasT_in[
                                            b, (jg * NB + ib) * P:
                                            (jg * NB + ib + 1) * P, :
                                        ],
                                    )
                            bgas[b] = bga
                        f2c = f2s[b][:, jg, jo, :]
                        nf2c = nf2s[b][:, jg, jo, :]
                        t2 = work.tile([P, NL], fp16, tag="t2")
                        nc.vector.tensor_scalar(
                            t2, nf1s[b], nf2c, 0.0, ALU.add, ALU.max
                        )
                        w = work.tile([P, NL], fp16, tag="w")
                        weng = nc.gpsimd if (jc % JC) < pool_w else nc.vector
                        weng.tensor_tensor(w, t2, bgas[b][:, jo, :], ALU.add)
                        e = work.tile([P, NL], fp16, tag="e")
                        nc.scalar.activation(e, w, AFT.Exp, bias=f2c,
                                             scale=(1.0 - SLOPE))
                        nc.tensor.matmul(
                            ps_ags[b],
                            lhsT=sfalls[b][:, jg, jo, 0:O + 1],
                            rhs=e,
                            start=(jc == 0), stop=(jc == JC - 1),
                        )

                # batched tails: softmax denominators -> DVE reciprocal
                rrows = []
                ssqs = []
                for b in range(B):
                    den = tailp.tile([1, NL], f32, tag="den")
                    nc.vector.tensor_copy(den, ps_ags[b][O:O + 1, :])
                    rrow = tailp.tile([1, NL], f32r, tag="rrow")
                    with nc.allow_low_precision(reason="recip feeds f32r bcast matmul"):
                        nc.vector.reciprocal(rrow, den)
                    rrows.append(rrow)
                ssums = []
                for b in range(B):
                    ps_bc = ps_misc.tile([O, NL], f32, tag="pmisc")
                    nc.tensor.matmul(ps_bc, lhsT=ones_o,
                                     rhs=rrows[b], start=True, stop=True)
                    bc = tailp.tile([O, NL], f32, tag="bc")
                    nc.vector.tensor_copy(bc, ps_bc)
                    ssum_b = tailp.tile([O, 1], f32, tag="ssum_b")
                    nc.vector.scalar_tensor_tensor(
                        valsT[:, b * NL:(b + 1) * NL], ps_ags[b][0:O, :], 0.0,
                        bc, ALU.add, ALU.mult, accum_out=ssum_b,
                    )
                    ssums.append(ssum_b)
                    sq_b = tailp.tile([O, NL], f32, tag="sq_b")
                    ssq_b = tailp.tile([O, 1], f32, tag="ssq_b")
                    nc.vector.scalar_tensor_tensor(
                        sq_b, valsT[:, b * NL:(b + 1) * NL], 0.0,
                        valsT[:, b * NL:(b + 1) * NL], ALU.add, ALU.mult,
                        accum_out=ssq_b,
                    )
                    ssqs.append(ssq_b)

                # ---------- BatchNorm stats + AllReduce ----------
                ssum = tailp.tile([O, 1], f32, tag="ssum")
                nc.vector.tensor_tensor(ssum, ssums[0], ssums[1], ALU.add)
                ssq = tailp.tile([O, 1], f32, tag="ssq")
                nc.vector.tensor_tensor(ssq, ssqs[0], ssqs[1], ALU.add)
                stt = tailp.tile([O, 2], f32, tag="stt")
                nc.vector.tensor_copy(stt[:, 0:1], ssum)
                nc.vector.tensor_copy(stt[:, 1:2], ssq)
                nc.sync.dma_start(out=st_in.ap(), in_=stt)
                if not no_cc_stats:
                    nc.gpsimd.collective_compute(
                        "AllGather", ALU.bypass, replica_groups=rg,
                        ins=[st_in.ap()], outs=[st_out.ap()],
                    )
                # gather per-rank partials [o, (sum,sumsq), rank] and reduce
                tot3 = tailp.tile([O, 2, R], f32, tag="tot3")
                nc.sync.dma_start(
                    out=tot3,
                    in_=bass.AP(
                        tensor=st_out.ap().tensor, offset=0,
                        ap=[[2, O], [1, 2], [2 * O, R]],
                    ),
                )
                tot = tailp.tile([O, 2], f32, tag="tot")
                nc.vector.tensor_reduce(
                    tot, tot3, axis=mybir.AxisListType.X, op=ALU.add
                )

                mean = tailp.tile([O, 1], f32, tag="mean")
                nc.vector.tensor_scalar_mul(mean, tot[:, 0:1], 1.0 / (B * N))
                ex2 = tailp.tile([O, 1], f32, tag="ex2")
                nc.vector.tensor_scalar_mul(ex2, tot[:, 1:2], 1.0 / (B * N))
                msq = tailp.tile([O, 1], f32, tag="msq")
                nc.vector.tensor_tensor(msq, mean, mean, ALU.mult)
                var = tailp.tile([O, 1], f32, tag="var")
                nc.vector.tensor_tensor(var, ex2, msq, ALU.subtract)
                lnv = tailp.tile([O, 1], f32, tag="lnv")
                nc.scalar.activation(lnv, var, AFT.Ln, bias=eps_t)
                istd = tailp.tile([O, 1], f32, tag="istd")
                nc.scalar.activation(istd, lnv, AFT.Exp, scale=-0.5)
                scal = tailp.tile([O, 1], f32, tag="scal")
                nc.vector.tensor_tensor(scal, istd, gam, ALU.mult)
                mscal = tailp.tile([O, 1], f32, tag="mscal")
                nc.vector.tensor_tensor(mscal, mean, scal, ALU.mult)
                shift = tailp.tile([O, 1], f32, tag="shift")
                nc.vector.tensor_tensor(shift, bet, mscal, ALU.subtract)

                ret = persist.tile([O, B * NL], f32, tag="ret")
                nc.vector.tensor_scalar(ret, valsT, scal, shift,
                                        ALU.mult, ALU.add)
                pos = persist.tile([O, B * NL], f32, tag="pos")
                nc.scalar.activation(pos, ret, AFT.Relu)
                mng = persist.tile([O, B * NL], f32, tag="mng")
                nc.vector.tensor_scalar_min(mng, ret, 0.0)
                em = persist.tile([O, B * NL], f32, tag="em")
                nc.scalar.activation(em, mng, AFT.Exp)
                fin = persist.tile([O, B * NL], f32, tag="fin")
                nc.vector.scalar_tensor_tensor(fin, pos, -1.0, em, ALU.add, ALU.add)

                # ---------- output transpose + store ----------
                for b in range(B):
                    for nb in range(NB):
                        c0 = b * NL + nb * P
                        ps_oT = ps_misc.tile([P, O], f32, tag="pmisc")
                        nc.tensor.transpose(ps_oT, fin[:, c0:c0 + P], ident[0:O, 0:O])
                        oT = stage.tile([P, O], f32, tag="oT")
                        nc.vector.tensor_copy(oT, ps_oT)
                        nc.sync.dma_start(
                            out=out_ext[b, nb * P:(nb + 1) * P, :], in_=oT
                        )

            def _dma_body():
                for b in range(B):
                    for jg in range(JG):
                        bga = biasg.tile([P, NB, NL], fp16, tag="biasg")
                        dma_engs = [nc.sync, nc.scalar] if bias_alt \
                            else [nc.sync, nc.sync]
                        for ib in range(NB):
                            dma_engs[(2 * b + ib) % 2].dma_start(
                                out=bga[:, ib, :],
                                in_=biasT_in[
                                    b, (jg * NB + ib) * P:(jg * NB + ib + 1) * P, :
                                ],
                            )
                    for nb in range(NB):
                        seq_t = stage.tile([P, F], f32, tag="seq_t")
                        nc.sync.dma_start(
                            out=seq_t, in_=seq_in[b, nb * P:(nb + 1) * P, :]
                        )
                for b in range(B):
                    sfall = perb.tile([P, R, NB, AGW], fp16, tag="sfall")
                    for rank in range(R):
                        nc.sync.dma_start(
                            out=sfall[:, rank, :, :],
                            in_=bass.AP(
                                tensor=ag_out.ap().tensor,
                                offset=(rank * B * NL + b * NL) * AGW,
                                ap=[[AGW, P], [P * AGW, NB], [1, AGW]],
                            ),
                        )
                zt = stage.tile([P, O], f32, tag="oT")
                nc.vector.memset(zt, 0.0)
                for b in range(B):
                    for nb in range(NB):
                        nc.sync.dma_start(
                            out=out_ext[b, nb * P:(nb + 1) * P, :], in_=zt
                        )

            for _rep in range(n_reps):
                if dma_only:
                    _dma_body()
                else:
                    _rep_body()

    nc.compile()
    _CACHE[key] = nc
    return nc


BF16_PROJ = True


def _prep_inputs(seq, bias_mat, W1, w2, b2, w3, b3, gamma, beta):
    seq_np = np.float16 if BF16_PROJ else np.float32
    seq = np.ascontiguousarray(seq, dtype=np.float32).astype(seq_np)
    bias_mat = np.asarray(bias_mat, dtype=np.float32)
    w1t = np.ascontiguousarray(
        np.asarray(W1, dtype=np.float32).T).astype(seq_np)
    w2c = np.asarray(w2, dtype=np.float32).reshape(O, 1)
    w3c = np.asarray(w3, dtype=np.float32).reshape(O, 1)
    gam = np.asarray(gamma, dtype=np.float32).reshape(O, 1)
    bet = np.asarray(beta, dtype=np.float32).reshape(O, 1)
    sc = np.array([[float(b2), float(b3)]], dtype=np.float32)

    in_maps = []
    for c in range(R):
        # host-side: transpose core's row-block to key-major [B, N, NL],
        # pre-scale by 1/(1-slope) (the ACT exp applies *(1-slope) on chip),
        # and convert to bf16.
        bT = bias_mat[:, c * NL:(c + 1) * NL, :].transpose(0, 2, 1)
        bT = np.ascontiguousarray(bT) * (1.0 / (1.0 - SLOPE))
        bT = bT.astype(np.float16)
        in_maps.append({
            "seq_loc": np.ascontiguousarray(seq[:, c * NL:(c + 1) * NL, :]),
            "biasT_loc": bT,
            "w1t": w1t,
            "w2c": w2c,
            "w3c": w3c,
            "gamma_c": gam,
            "beta_c": bet,
            "scalars": sc,
        })
    return in_maps


def kernel(seq, bias_mat, W1, w2, b2, w3, b3, gamma, beta):
    in_maps = _prep_inputs(seq, bias_mat, W1, w2, b2, w3, b3, gamma, beta)
    nc = _build_program(bf16_proj=BF16_PROJ)
    res = run_bass_kernel_spmd(nc, in_maps, core_ids=list(range(R)))
    out = np.concatenate([res.results[c]["out_loc"] for c in range(R)], axis=1)
    return out
